# revision 38
# baseline (speedup 1.0000x reference)
"""MoNet (2-layer GMMConv) Trainium2 Bass kernel -- self-contained.

Edge parallelism with src-half core pairing, dst-quarter sharding, and a
degree-sorted ELL edge layout:
 - Edges split by src half (fits int16 gather indices with NO per-node
   half split); lo-half edges go to cores 0-3, hi-half to cores 4-7,
   sharded by dst quarter. Cores c and c+4 cover the same dst range;
   the host sums their partial outputs (and adds the bias).
 - Per core, dst nodes are sorted by degree and packed into 128-node
   tiles; each tile's edges sit in C_t ELL columns (column c = c-th
   edge of each node). The segment sum is then a pure free-dim
   reduction on the Vector engine -- NO one-hot build and NO PE
   matmuls in the edge phase (the PE only builds the half gather
   table). C_t profile is unified across cores so one kernel serves
   all 8.
 - Device per layer: (A) hp = x @ W.T bf16 half-table build; (W)
   per-edge mixture weights w[e,k] = exp(a_k + b.p + c.p^2) masked at
   ELL pad slots; (B) batched dma_gather of hp[src] rows, DVE
   weight-fold into a (out, col, k)-major message buffer, per-tile
   X-axis tensor_reduce over (col,k), f32 combine, row-major output.
"""

import sys
import numpy as np
import ml_dtypes

for p in ("/opt/trn_rl_repo",):
    if p not in sys.path:
        sys.path.insert(0, p)

import concourse.bass as bass
import concourse.mybir as mybir
import concourse.tile as tile
from concourse import bacc
from concourse import bass_utils

F32 = mybir.dt.float32
BF16 = mybir.dt.bfloat16
I32 = mybir.dt.int32
I16 = mybir.dt.int16

P = 128
NPBF = ml_dtypes.bfloat16


# ----------------------------------------------------------------------------
# Host-side metadata (index preprocessing / sharding)
# ----------------------------------------------------------------------------

def build_ell_metadata(src, dst, pseudo, n_nodes, n_cores=8, G=16):
    """Split edges by src half (core pairing) and dst quarter (sharding),
    degree-sort dst nodes per core, pack edges into a unified-profile ELL
    layout, and build the wrapped int16 gather-index array."""
    assert n_cores == 8
    NT = ((-(-n_nodes // P)) + 7) // 8 * 8          # tiles, multiple of 8
    NPAD = NT * P
    HALF = NPAD // 2
    T4 = NT // 4
    QN = T4 * P

    E = len(src)
    is_hi = (src >= HALF).astype(np.int64)
    q = dst // QN
    core = is_hi * 4 + q
    dloc = dst - q * QN

    deg = np.zeros((n_cores, QN), np.int64)
    np.add.at(deg, (core, dloc), 1)
    order = np.argsort(-deg, axis=1, kind="stable")     # (8, QN) node ids
    pos = np.empty((n_cores, QN), np.int64)
    rows = np.arange(n_cores)[:, None]
    pos[rows, order] = np.arange(QN)[None, :]
    degsorted = np.take_along_axis(deg, order, axis=1)
    C_uni = degsorted.reshape(n_cores, T4, P)[:, :, 0].max(axis=0)  # (T4,)
    colbase = np.zeros(T4 + 1, np.int64)
    np.cumsum(C_uni, out=colbase[1:])
    TCE = int(colbase[-1])

    # rank of each edge within its (core, dst) group
    key = core * np.int64(QN) + dloc
    eorder = np.argsort(key, kind="stable")
    skey = key[eorder]
    starts = np.zeros(E, np.int64)
    newgrp = np.empty(E, np.bool_)
    newgrp[0] = True
    newgrp[1:] = skey[1:] != skey[:-1]
    gidx = np.cumsum(newgrp) - 1
    gstart = np.flatnonzero(newgrp)
    rank = np.empty(E, np.int64)
    rank[eorder] = np.arange(E) - gstart[gidx]

    epos = pos[core, dloc]
    tl = epos // P
    pp = epos % P
    col = colbase[tl] + rank

    src_loc = (src - is_hi * HALF).astype(np.int16)
    src_t = np.zeros((n_cores, P, TCE), np.int16)
    mask_t = np.zeros((n_cores, P, TCE), np.float32)
    ps_a = np.zeros((n_cores, P, TCE), np.float32)
    ps_b = np.zeros((n_cores, P, TCE), np.float32)
    src_t[core, pp, col] = src_loc
    mask_t[core, pp, col] = 1.0
    ps_a[core, pp, col] = pseudo[:, 0]
    ps_b[core, pp, col] = pseudo[:, 1]
    maskd_t = np.repeat(mask_t, 2, axis=2).astype(NPBF)

    nb = -(-TCE // G)
    SB = P * G // 16
    idx_w = np.zeros((n_cores, P, nb * SB), np.int16)
    for b in range(nb):
        c0 = b * G
        gn = min(G, TCE - c0)
        flat = src_t[:, :, c0:c0 + gn].transpose(0, 2, 1).reshape(
            n_cores, gn * P)
        w = flat.reshape(n_cores, gn * P // 16, 16).transpose(0, 2, 1)
        idx_w[:, :, b * SB:b * SB + gn * P // 16] = np.tile(w, (1, 8, 1))

    return dict(idx_w=idx_w, maskd_t=maskd_t, ps_a=ps_a, ps_b=ps_b,
                order=order, NT=NT, T4=T4, QN=QN, HALF=HALF,
                C_uni=tuple(int(x) for x in C_uni), TCE=TCE, G=G)


def pack_params(pp_w, pp_b, mu, inv_sigma):
    """Fold the Gaussian-mixture parameters into the quadratic form
    logw_k = a_k + sum_d b_kd p_d + c_kd p_d^2 (parameter-only algebra)."""
    K = mu.shape[0]
    is2 = inv_sigma.astype(np.float64) ** 2
    a = -0.5 * (is2 * mu.astype(np.float64) ** 2).sum(axis=1)
    b = is2 * mu
    c = -0.5 * is2
    par = np.zeros(32, np.float32)
    par[0] = pp_w[0, 0]; par[1] = pp_w[0, 1]
    par[2] = pp_w[1, 0]; par[3] = pp_w[1, 1]
    par[4] = pp_b[0]; par[5] = pp_b[1]
    par[6:6 + K] = a
    par[9:9 + K] = b[:, 0]
    par[12:12 + K] = b[:, 1]
    par[15:15 + K] = c[:, 0]
    par[18:18 + K] = c[:, 1]
    return par.reshape(1, 32)


# ----------------------------------------------------------------------------
# Device kernel builder (one GMMConv layer, one src-half, one dst-quarter)
# ----------------------------------------------------------------------------

def build_layer_kernel(T4, HALF, C_uni, IN_C, OUT, K=3, G=16, n_cores=8,
                       gat_bufs=10):
    """One GMMConv layer on a src-half table with ELL edge layout."""
    ROWF = K * OUT
    ROWP = -(-ROWF // 128) * 128        # bf16 row pad to 256B multiple
    NTH = HALF // P                     # tiles in the half table
    TCE = sum(C_uni)
    colbase = [0]
    for c in C_uni:
        colbase.append(colbase[-1] + c)
    nb = -(-TCE // G)
    SB = P * G // 16
    NIDXCOLS = nb * SB
    F2 = OUT // 2

    # even-index non-empty tiles take the PE transpose-accumulate path
    pe_tiles = [t for t in range(T4) if C_uni[t] > 0 and t % 3 < 2]
    pe_slot = {t: j for j, t in enumerate(pe_tiles)}
    NPE = max(1, len(pe_tiles))

    nc = bacc.Bacc("TRN2", target_bir_lowering=False, debug=False,
                   num_devices=n_cores, num_swdge_queues=4)
    xT = nc.dram_tensor("xT", [IN_C, HALF], BF16, kind="ExternalInput")
    wT = nc.dram_tensor("wT", [IN_C, ROWF], BF16, kind="ExternalInput")
    idx_d = nc.dram_tensor("idx_w", [P, NIDXCOLS], I16, kind="ExternalInput")
    msk_d = nc.dram_tensor("maskd_t", [P, 2 * TCE], BF16,
                           kind="ExternalInput")
    psa_d = nc.dram_tensor("ps_a", [P, TCE], F32, kind="ExternalInput")
    psb_d = nc.dram_tensor("ps_b", [P, TCE], F32, kind="ExternalInput")
    par_d = nc.dram_tensor("par", [1, 32], F32, kind="ExternalInput")
    hout_d = nc.dram_tensor("h_out", [T4 * P, OUT], F32,
                            kind="ExternalOutput")
    houtpe_d = nc.dram_tensor("h_out_pe", [OUT, NPE * P], F32,
                              kind="ExternalOutput")
    hp_d = nc.dram_tensor("hp", [HALF, ROWP], BF16)

    with tile.TileContext(nc) as tc:
        with (
            tc.tile_pool(name="const", bufs=1) as cst,
            tc.tile_pool(name="gat", bufs=gat_bufs) as gatp,
            tc.tile_pool(name="msg", bufs=5) as msgp,
            tc.tile_pool(name="red", bufs=8) as redp,
            tc.tile_pool(name="hrow", bufs=8) as hrowp,
            tc.tile_pool(name="psB", bufs=4, space="PSUM") as psB,
        ):
            # ---- constants / inputs ----
            wTs = cst.tile([IN_C, ROWF], BF16)
            nc.sync.dma_start(out=wTs[:], in_=wT[:])
            spar = cst.tile([P, 32], F32)
            nc.sync.dma_start(out=spar[:], in_=par_d[:].to_broadcast((P, 32)))
            # identity matrix for PE transpose-accumulate
            iota_i = cst.tile([P, P], I32)
            nc.gpsimd.iota(iota_i[:], pattern=[[1, P]], base=0,
                           channel_multiplier=0)
            rowid_i = cst.tile([P, P], I32)
            nc.gpsimd.iota(rowid_i[:], pattern=[[0, P]], base=0,
                           channel_multiplier=1)
            ident = cst.tile([P, P], BF16)
            nc.vector.tensor_tensor(ident[:], iota_i[:], rowid_i[:],
                                    op=mybir.AluOpType.is_equal)

            # ---- inputs for the edge phase ----
            idx_s = cst.tile([P, NIDXCOLS], I16)
            nc.sync.dma_start(out=idx_s[:], in_=idx_d[:])

            # ---- phase W head: tanh projections (issued before phase A so
            #      only two scalar-queue ops precede A's PSUM copies; the
            #      DVE-heavy tail is issued after A and overlaps it) ----
            def ts_mul(out, in0, j):
                nc.vector.tensor_scalar_mul(out, in0, spar[:, j:j + 1])

            wkd = [cst.tile([P, 2 * TCE], BF16, name=f"wkd{k}", tag=f"wd{k}")
                   for k in range(K)]
            mskd_s = cst.tile([P, 2 * TCE], BF16)
            nc.sync.dma_start(out=mskd_s[:], in_=msk_d[:])
            with tc.tile_pool(name="wprep", bufs=1) as wpp:
                psa_s = wpp.tile([P, TCE], F32, tag="psa")
                nc.sync.dma_start(out=psa_s[:], in_=psa_d[:])
                psb_s = wpp.tile([P, TCE], F32, tag="psb")
                nc.sync.dma_start(out=psb_s[:], in_=psb_d[:])
                pa = wpp.tile([P, TCE], F32, tag="pa")
                pb = wpp.tile([P, TCE], F32, tag="pb")
                m1 = wpp.tile([P, TCE], F32, tag="m1")
                m2 = wpp.tile([P, TCE], F32, tag="m2")
                m3 = wpp.tile([P, TCE], F32, tag="m3")
                m4 = wpp.tile([P, TCE], F32, tag="m4")
                ts_mul(m1[:], psa_s[:], 0)
                ts_mul(m2[:], psb_s[:], 1)
                nc.vector.tensor_add(m1[:], m1[:], m2[:])
                ts_mul(m3[:], psa_s[:], 2)
                ts_mul(m4[:], psb_s[:], 3)
                nc.vector.tensor_add(m3[:], m3[:], m4[:])
                nc.scalar.activation(pa[:], m1[:],
                                     mybir.ActivationFunctionType.Tanh,
                                     bias=spar[:, 4:5])
                nc.scalar.activation(pb[:], m3[:],
                                     mybir.ActivationFunctionType.Tanh,
                                     bias=spar[:, 5:6])

                # ---- phase A: hp half-table build (bf16 table) ----
                BLKT = 8
                nblk = -(-NTH // BLKT)
                with (
                    tc.tile_pool(name="xblk", bufs=3) as xblkp,
                    tc.tile_pool(name="hps", bufs=4) as hpsp,
                    tc.tile_pool(name="psA", bufs=4, space="PSUM") as psA,
                ):
                    for b in range(nblk):
                        t0 = b * BLKT
                        tn = min(BLKT, NTH - t0)
                        xblk = xblkp.tile([IN_C, BLKT * P], BF16, tag="xblk")
                        nc.sync.dma_start(out=xblk[:, :tn * P],
                                          in_=xT[:, t0 * P:(t0 + tn) * P])
                        hps = hpsp.tile([P, BLKT * ROWF], BF16, tag="hps")
                        for i0 in range(0, tn, 2):
                            pn = min(2, tn - i0)
                            pst = psA.tile([P, 2 * ROWF], F32)
                            for i in range(i0, i0 + pn):
                                nc.tensor.matmul(
                                    pst[:,
                                        (i - i0) * ROWF:(i - i0 + 1) * ROWF],
                                    lhsT=xblk[:, i * P:(i + 1) * P],
                                    rhs=wTs[:], start=True, stop=True)
                            nc.scalar.activation(
                                hps[:, i0 * ROWF:(i0 + pn) * ROWF],
                                pst[:, :pn * ROWF],
                                mybir.ActivationFunctionType.Copy)
                        r0 = t0 * P
                        nc.sync.dma_start(
                            out=hp_d[r0:r0 + tn * P, 0:ROWF].rearrange(
                                "(g p) f -> p g f", p=P),
                            in_=hps[:, :tn * ROWF].rearrange(
                                "p (g f) -> p g f", f=ROWF))

                # ---- phase W tail: quadratic form + exp (overlaps A) ----
                qa = wpp.tile([P, TCE], F32, tag="psa")
                qb = wpp.tile([P, TCE], F32, tag="psb")
                nc.vector.tensor_tensor(qa[:], pa[:], pa[:],
                                        op=mybir.AluOpType.mult)
                nc.vector.tensor_tensor(qb[:], pb[:], pb[:],
                                        op=mybir.AluOpType.mult)
                for k in range(K):
                    u1 = wpp.tile([P, TCE], F32, tag="m1", name=f"u1_{k}")
                    u2 = wpp.tile([P, TCE], F32, tag="m2", name=f"u2_{k}")
                    u3 = wpp.tile([P, TCE], F32, tag="m3", name=f"u3_{k}")
                    u4 = wpp.tile([P, TCE], F32, tag="m4", name=f"u4_{k}")
                    ts_mul(u1[:], pa[:], 9 + k)
                    ts_mul(u2[:], pb[:], 12 + k)
                    ts_mul(u3[:], qa[:], 15 + k)
                    ts_mul(u4[:], qb[:], 18 + k)
                    nc.vector.tensor_add(u1[:], u1[:], u2[:])
                    nc.vector.tensor_add(u3[:], u3[:], u4[:])
                    nc.vector.tensor_add(u1[:], u1[:], u3[:])
                    # exp with pair-duplicated output, then pad-mask fold
                    nc.scalar.activation(
                        wkd[k][:].rearrange("p (c two) -> p c two", two=2),
                        u1[:].to_broadcast((P, TCE, 2)),
                        mybir.ActivationFunctionType.Exp,
                        bias=spar[:, 6 + k:7 + k])
                    nc.vector.tensor_tensor(
                        out=wkd[k][:], in0=wkd[k][:], in1=mskd_s[:],
                        op=mybir.AluOpType.mult)

            # ---- phase B: gather + fold + k-sum + per-tile add-tree ----
            sum_tiles = {}
            gat_tiles = {}
            LOOKAHEAD = 6

            def prefetch_gather(b):
                """Issue the gather only (gpsimd queue) -- deep prefetch
                without putting gather-dependent folds in the DVE queue."""
                if b in gat_tiles or b >= nb:
                    return
                c0 = b * G
                gn = min(G, TCE - c0)
                gat = gatp.tile([P, G * ROWP], BF16, tag="gat",
                                name=f"gat{b}")
                gv = gat[:].rearrange("p (j f) -> p j f", f=ROWP)
                nsplit = 4 if (b < 4 or b >= nb - 3) and gn % 4 == 0 \
                    else (2 if gn % 2 == 0 else 1)
                qn = gn // nsplit
                for qq in range(nsplit):
                    nc.gpsimd.dma_gather(
                        out_ap=gv[:, qq * qn:(qq + 1) * qn, :],
                        in_ap=hp_d[:, :],
                        idxs_ap=idx_s[:, b * SB + qq * qn * P // 16:
                                      b * SB + (qq + 1) * qn * P // 16],
                        num_idxs=qn * P, num_idxs_reg=qn * P,
                        elem_size=ROWP, single_packet=False,
                        queue_num=(2 * b + qq) % 4)
                gat_tiles[b] = gat

            def issue_batch(b):
                if b in sum_tiles:
                    return sum_tiles[b]
                prefetch_gather(b)
                prefetch_gather(b + LOOKAHEAD)
                c0 = b * G
                gn = min(G, TCE - c0)
                gat = gat_tiles.pop(b)
                gat3 = gat[:].rearrange("p (j f) -> p j f", f=ROWP)
                mks = []
                for k in range(K):
                    mk = msgp.tile([P, G * OUT], BF16, tag=f"mk{k}",
                                   name=f"mk{k}_{b}")
                    nc.vector.tensor_tensor(
                        out=mk[:].rearrange("p (j f2 two) -> p j f2 two",
                                            f2=F2, two=2)[:, :gn, :, :],
                        in0=gat3[:, :gn, k * OUT:(k + 1) * OUT].rearrange(
                            "p j (f2 two) -> p j f2 two", two=2),
                        in1=wkd[k][:, 2 * c0:2 * (c0 + gn)].rearrange(
                            "p (c two) -> p c two", two=2).unsqueeze(
                            2).broadcast_to((P, gn, F2, 2)),
                        op=mybir.AluOpType.mult)
                    mks.append(mk)
                # k-sum: sum3[p, c, o] = mk0 + mk1 + mk2
                sum3 = msgp.tile([P, G * OUT], BF16, tag="sum3",
                                 name=f"sum3_{b}")
                nc.vector.tensor_add(sum3[:, :gn * OUT], mks[0][:, :gn * OUT],
                                     mks[1][:, :gn * OUT])
                nc.vector.tensor_add(sum3[:, :gn * OUT], sum3[:, :gn * OUT],
                                     mks[2][:, :gn * OUT])
                sum_tiles[b] = sum3
                return sum3

            def col_view(buf, a, n):
                return buf[:, a * OUT:(a + n) * OUT]

            def seg_tree(t, s0, s1, out_f32):
                """Tree-sum sum3 columns [s0,s1) of one batch.
                If out_f32 is not None, the final add writes it (f32) and
                returns None; otherwise returns a (P, OUT) bf16 part."""
                b = s0 // G
                sum3 = issue_batch(b)
                src, a, n = sum3, s0 - b * G, s1 - s0
                while n > 2:
                    h = n // 2
                    scr = redp.tile([P, (G // 2) * OUT], BF16, tag="scr",
                                    name=f"scr{t}_{s0}_{n}")
                    nc.vector.tensor_add(col_view(scr, 0, h),
                                         col_view(src, a, h),
                                         col_view(src, a + h, h))
                    if n % 2:
                        nc.vector.tensor_add(col_view(scr, 0, 1),
                                             col_view(scr, 0, 1),
                                             col_view(src, a + 2 * h, 1))
                    src, a, n = scr, 0, h
                if n == 2:
                    if out_f32 is not None:
                        nc.vector.tensor_add(out_f32[:],
                                             col_view(src, a, 1),
                                             col_view(src, a + 1, 1))
                        return None
                    part = redp.tile([P, OUT], BF16, tag="part",
                                     name=f"part{t}_{s0}")
                    nc.vector.tensor_add(part[:], col_view(src, a, 1),
                                         col_view(src, a + 1, 1))
                    return (part, 0)
                # n == 1
                if out_f32 is not None:
                    nc.vector.tensor_scalar(
                        out=out_f32[:], in0=col_view(src, a, 1),
                        scalar1=0.0, scalar2=None, op0=mybir.AluOpType.add)
                    return None
                return (src, a)

            for b in range(LOOKAHEAD):
                prefetch_gather(b)
            for t in range(T4):
                cb0, cb1 = colbase[t], colbase[t + 1]
                if t in pe_slot:
                    # PE path: PSUM[o, p] += sum3_col[p, o] via identity rhs
                    ps = psB.tile([OUT, P], F32, tag="peacc",
                                  name=f"pe{t}")
                    ncols = cb1 - cb0
                    for ci in range(ncols):
                        col = cb0 + ci
                        b = col // G
                        sum3 = issue_batch(b)
                        nc.tensor.matmul(
                            ps[:], lhsT=col_view(sum3, col - b * G, 1),
                            rhs=ident[:], start=(ci == 0),
                            stop=(ci == ncols - 1))
                    pe_sb = hrowp.tile([OUT, P], F32, tag="pesb",
                                       name=f"pesb{t}")
                    nc.scalar.activation(pe_sb[:], ps[:],
                                         mybir.ActivationFunctionType.Copy)
                    j = pe_slot[t]
                    nc.sync.dma_start(out=houtpe_d[:, j * P:(j + 1) * P],
                                      in_=pe_sb[:])
                    continue
                hrow = hrowp.tile([P, OUT], F32, tag="hrow",
                                  name=f"hrow{t}")
                segs = []
                s = cb0
                while s < cb1:
                    e = min(cb1, (s // G + 1) * G)
                    segs.append((s, e))
                    s = e
                if not segs:
                    nc.vector.memset(hrow[:], 0.0)
                elif len(segs) == 1:
                    seg_tree(t, segs[0][0], segs[0][1], hrow)
                else:
                    parts = [seg_tree(t, s0, s1, None) for s0, s1 in segs]
                    buf0, a0 = parts[0]
                    buf1, a1 = parts[1]
                    nc.vector.tensor_add(hrow[:], col_view(buf0, a0, 1),
                                         col_view(buf1, a1, 1))
                    for bufx, ax in parts[2:]:
                        nc.vector.tensor_add(hrow[:], hrow[:],
                                             col_view(bufx, ax, 1))
                nc.sync.dma_start(out=hout_d[t * P:(t + 1) * P, :],
                                  in_=hrow[:])

    nc.compile()
    return nc


# ----------------------------------------------------------------------------
# Full model runner
# ----------------------------------------------------------------------------

_KERNEL_CACHE = {}


def _get_kernel(key, builder):
    if key not in _KERNEL_CACHE:
        _KERNEL_CACHE[key] = builder()
    return _KERNEL_CACHE[key]


def _run_layer(layer_tag, md, xfull_T, fcT, par, bias, IN_C, OUT,
               n_cores, trace):
    """xfull_T: (IN_C, NPAD) bf16 full-node feature table (feature-major).
    Returns (NPAD, OUT) f32 node outputs (bias added)."""
    T4, HALF, QN = md["T4"], md["HALF"], md["QN"]
    C_uni, G = md["C_uni"], md["G"]
    nc = _get_kernel((layer_tag, T4, HALF, C_uni, IN_C, OUT, G, n_cores),
                     lambda: build_layer_kernel(T4, HALF, C_uni, IN_C, OUT,
                                                3, G, n_cores))
    in_maps = []
    for c in range(n_cores):
        h = c // 4
        in_maps.append(dict(
            xT=np.ascontiguousarray(xfull_T[:, h * HALF:(h + 1) * HALF]),
            wT=fcT,
            idx_w=md["idx_w"][c], maskd_t=md["maskd_t"][c],
            ps_a=md["ps_a"][c], ps_b=md["ps_b"][c],
            par=par))
    res = bass_utils.run_bass_kernel_spmd(
        nc, in_maps, core_ids=list(range(n_cores)), trace=trace)

    NPAD = 2 * HALF
    pe_tiles = [t for t in range(T4) if C_uni[t] > 0 and t % 3 < 2]
    out = np.zeros((NPAD, OUT), np.float32)
    order = md["order"]
    for c in range(8):
        harr = np.array(res.results[c]["h_out"])
        hpe = res.results[c]["h_out_pe"]
        for j, t in enumerate(pe_tiles):
            harr[t * P:(t + 1) * P] = hpe[:, j * P:(j + 1) * P].T
        base = (c % 4) * QN
        out[base + order[c]] += harr
    out += bias.reshape(1, OUT)
    return out, res.exec_time_ns


def run_monet(inputs, n_cores=8, G=16, trace=False):
    feat = np.asarray(inputs["feat"], np.float32)
    pseudo = np.asarray(inputs["pseudo"], np.float32)
    src = np.asarray(inputs["src"], np.int32)
    dst = np.asarray(inputs["dst"], np.int32)
    N, IN_F = feat.shape
    HID = np.asarray(inputs["fc0"]).shape[0] // 3
    OUTF = np.asarray(inputs["fc1"]).shape[0] // 3

    md = build_ell_metadata(src, dst, pseudo, N, n_cores, G)
    NPAD = md["NT"] * P

    featT = np.zeros((IN_F, NPAD), NPBF)
    featT[:, :N] = feat.T.astype(NPBF)
    fc0T = np.ascontiguousarray(
        np.asarray(inputs["fc0"], np.float32).T.astype(NPBF))
    fc1T = np.ascontiguousarray(
        np.asarray(inputs["fc1"], np.float32).T.astype(NPBF))
    par0 = pack_params(np.asarray(inputs["pp0_w"], np.float32),
                       np.asarray(inputs["pp0_b"], np.float32),
                       np.asarray(inputs["mu0"], np.float32),
                       np.asarray(inputs["inv_sigma0"], np.float32))
    par1 = pack_params(np.asarray(inputs["pp1_w"], np.float32),
                       np.asarray(inputs["pp1_b"], np.float32),
                       np.asarray(inputs["mu1"], np.float32),
                       np.asarray(inputs["inv_sigma1"], np.float32))
    b0 = np.asarray(inputs["b0"], np.float32)
    b1 = np.asarray(inputs["b1"], np.float32)

    h0, t0 = _run_layer("l0v4", md, featT, fc0T, par0, b0, IN_F, HID,
                        n_cores, trace)

    hT = np.ascontiguousarray(h0.T.astype(NPBF))
    h1, t1 = _run_layer("l1v4", md, hT, fc1T, par1, b1, HID, OUTF,
                        n_cores, trace)

    out = np.ascontiguousarray(h1[:N])
    perf = dict(l0_ns=t0, l1_ns=t1)
    return out, perf


# ----------------------------------------------------------------------------
# Harness entry: full inputs in, full output out
# ----------------------------------------------------------------------------

def kernel(**inputs):
    out, _ = run_monet(inputs)
    return out.astype(np.float32)


# revision 42
# speedup vs baseline: 1.0443x; 1.0443x over previous
"""MoNet (2-layer GMMConv) Trainium2 Bass kernel -- self-contained.

Edge parallelism with src-half core pairing, dst-quarter sharding, and a
degree-sorted ELL edge layout:
 - Edges split by src half (fits int16 gather indices with NO per-node
   half split); lo-half edges go to cores 0-3, hi-half to cores 4-7,
   sharded by dst quarter. Cores c and c+4 cover the same dst range;
   the host sums their partial outputs (and adds the bias).
 - Per core, dst nodes are sorted by degree and packed into 128-node
   tiles; each tile's edges sit in C_t ELL columns (column c = c-th
   edge of each node). The segment sum is then a pure free-dim
   reduction on the Vector engine -- NO one-hot build and NO PE
   matmuls in the edge phase (the PE only builds the half gather
   table). C_t profile is unified across cores so one kernel serves
   all 8.
 - Device per layer: (A) hp = x @ W.T bf16 half-table build; (W)
   per-edge mixture weights w[e,k] = exp(a_k + b.p + c.p^2) masked at
   ELL pad slots; (B) batched dma_gather of hp[src] rows, DVE
   weight-fold into a (out, col, k)-major message buffer, per-tile
   X-axis tensor_reduce over (col,k), f32 combine, row-major output.
"""

import sys
import numpy as np
import ml_dtypes

for p in ("/opt/trn_rl_repo",):
    if p not in sys.path:
        sys.path.insert(0, p)

import concourse.bass as bass
import concourse.mybir as mybir
import concourse.tile as tile
from concourse import bacc
from concourse import bass_utils

F32 = mybir.dt.float32
BF16 = mybir.dt.bfloat16
I32 = mybir.dt.int32
I16 = mybir.dt.int16

P = 128
NPBF = ml_dtypes.bfloat16


# ----------------------------------------------------------------------------
# Host-side metadata (index preprocessing / sharding)
# ----------------------------------------------------------------------------

def build_ell_metadata(src, dst, pseudo, n_nodes, n_cores=8, G=16):
    """Split edges by src half (core pairing) and dst quarter (sharding),
    degree-sort dst nodes per core, pack edges into a unified-profile ELL
    layout, and build the wrapped int16 gather-index array."""
    assert n_cores == 8
    NT = ((-(-n_nodes // P)) + 7) // 8 * 8          # tiles, multiple of 8
    NPAD = NT * P
    HALF = NPAD // 2
    T4 = NT // 4
    QN = T4 * P

    E = len(src)
    is_hi = (src >= HALF).astype(np.int64)
    q = dst // QN
    core = is_hi * 4 + q
    dloc = dst - q * QN

    deg = np.zeros((n_cores, QN), np.int64)
    np.add.at(deg, (core, dloc), 1)
    order = np.argsort(-deg, axis=1, kind="stable")     # (8, QN) node ids
    pos = np.empty((n_cores, QN), np.int64)
    rows = np.arange(n_cores)[:, None]
    pos[rows, order] = np.arange(QN)[None, :]
    degsorted = np.take_along_axis(deg, order, axis=1)
    C_uni = degsorted.reshape(n_cores, T4, P)[:, :, 0].max(axis=0)  # (T4,)
    colbase = np.zeros(T4 + 1, np.int64)
    np.cumsum(C_uni, out=colbase[1:])
    TCE = int(colbase[-1])

    # rank of each edge within its (core, dst) group
    key = core * np.int64(QN) + dloc
    eorder = np.argsort(key, kind="stable")
    skey = key[eorder]
    starts = np.zeros(E, np.int64)
    newgrp = np.empty(E, np.bool_)
    newgrp[0] = True
    newgrp[1:] = skey[1:] != skey[:-1]
    gidx = np.cumsum(newgrp) - 1
    gstart = np.flatnonzero(newgrp)
    rank = np.empty(E, np.int64)
    rank[eorder] = np.arange(E) - gstart[gidx]

    epos = pos[core, dloc]
    tl = epos // P
    pp = epos % P
    col = colbase[tl] + rank

    src_loc = (src - is_hi * HALF).astype(np.int16)
    src_t = np.zeros((n_cores, P, TCE), np.int16)
    mask_t = np.zeros((n_cores, P, TCE), np.float32)
    ps_a = np.zeros((n_cores, P, TCE), np.float32)
    ps_b = np.zeros((n_cores, P, TCE), np.float32)
    src_t[core, pp, col] = src_loc
    mask_t[core, pp, col] = 1.0
    ps_a[core, pp, col] = pseudo[:, 0]
    ps_b[core, pp, col] = pseudo[:, 1]
    maskd_t = np.repeat(mask_t, 2, axis=2).astype(NPBF)

    nb = -(-TCE // G)
    SB = P * G // 16
    idx_w = np.zeros((n_cores, P, nb * SB), np.int16)
    for b in range(nb):
        c0 = b * G
        gn = min(G, TCE - c0)
        flat = src_t[:, :, c0:c0 + gn].transpose(0, 2, 1).reshape(
            n_cores, gn * P)
        w = flat.reshape(n_cores, gn * P // 16, 16).transpose(0, 2, 1)
        idx_w[:, :, b * SB:b * SB + gn * P // 16] = np.tile(w, (1, 8, 1))

    return dict(idx_w=idx_w, maskd_t=maskd_t, ps_a=ps_a, ps_b=ps_b,
                order=order, NT=NT, T4=T4, QN=QN, HALF=HALF,
                C_uni=tuple(int(x) for x in C_uni), TCE=TCE, G=G)


def pack_params(pp_w, pp_b, mu, inv_sigma):
    """Fold the Gaussian-mixture parameters into the quadratic form
    logw_k = a_k + sum_d b_kd p_d + c_kd p_d^2 (parameter-only algebra)."""
    K = mu.shape[0]
    is2 = inv_sigma.astype(np.float64) ** 2
    a = -0.5 * (is2 * mu.astype(np.float64) ** 2).sum(axis=1)
    b = is2 * mu
    c = -0.5 * is2
    par = np.zeros(32, np.float32)
    par[0] = pp_w[0, 0]; par[1] = pp_w[0, 1]
    par[2] = pp_w[1, 0]; par[3] = pp_w[1, 1]
    par[4] = pp_b[0]; par[5] = pp_b[1]
    par[6:6 + K] = a
    par[9:9 + K] = b[:, 0]
    par[12:12 + K] = b[:, 1]
    par[15:15 + K] = c[:, 0]
    par[18:18 + K] = c[:, 1]
    return par.reshape(1, 32)


# ----------------------------------------------------------------------------
# Device kernel builder (one GMMConv layer, one src-half, one dst-quarter)
# ----------------------------------------------------------------------------

def build_layer_kernel(T4, HALF, C_uni, IN_C, OUT, K=3, G=16, n_cores=8,
                       gat_bufs=10):
    """One GMMConv layer on a src-half table with ELL edge layout."""
    ROWF = K * OUT
    ROWP = -(-ROWF // 128) * 128        # bf16 row pad to 256B multiple
    NTH = HALF // P                     # tiles in the half table
    TCE = sum(C_uni)
    colbase = [0]
    for c in C_uni:
        colbase.append(colbase[-1] + c)
    nb = -(-TCE // G)
    SB = P * G // 16
    NIDXCOLS = nb * SB
    F2 = OUT // 2

    # even-index non-empty tiles take the PE transpose-accumulate path
    pe_tiles = [t for t in range(T4) if C_uni[t] > 0 and t % 3 < 2]
    pe_slot = {t: j for j, t in enumerate(pe_tiles)}
    NPE = max(1, len(pe_tiles))

    nc = bacc.Bacc("TRN2", target_bir_lowering=False, debug=False,
                   num_devices=n_cores, num_swdge_queues=4)
    xT = nc.dram_tensor("xT", [IN_C, HALF], BF16, kind="ExternalInput")
    wT = nc.dram_tensor("wT", [IN_C, ROWF], BF16, kind="ExternalInput")
    idx_d = nc.dram_tensor("idx_w", [P, NIDXCOLS], I16, kind="ExternalInput")
    msk_d = nc.dram_tensor("maskd_t", [P, 2 * TCE], BF16,
                           kind="ExternalInput")
    psa_d = nc.dram_tensor("ps_a", [P, TCE], F32, kind="ExternalInput")
    psb_d = nc.dram_tensor("ps_b", [P, TCE], F32, kind="ExternalInput")
    par_d = nc.dram_tensor("par", [1, 32], F32, kind="ExternalInput")
    hout_d = nc.dram_tensor("h_out", [T4 * P, OUT], F32,
                            kind="ExternalOutput")
    houtpe_d = nc.dram_tensor("h_out_pe", [OUT, NPE * P], F32,
                              kind="ExternalOutput")
    hp_d = nc.dram_tensor("hp", [HALF, ROWP], BF16)

    with tile.TileContext(nc) as tc:
        with (
            tc.tile_pool(name="const", bufs=1) as cst,
            tc.tile_pool(name="gat", bufs=gat_bufs) as gatp,
            tc.tile_pool(name="msg", bufs=5) as msgp,
            tc.tile_pool(name="red", bufs=8) as redp,
            tc.tile_pool(name="hrow", bufs=8) as hrowp,
            tc.tile_pool(name="psB", bufs=4, space="PSUM") as psB,
        ):
            # ---- constants / inputs ----
            wTs = cst.tile([IN_C, ROWF], BF16)
            nc.sync.dma_start(out=wTs[:], in_=wT[:])
            spar = cst.tile([P, 32], F32)
            nc.sync.dma_start(out=spar[:], in_=par_d[:].to_broadcast((P, 32)))
            # identity matrix for PE transpose-accumulate
            iota_i = cst.tile([P, P], I32)
            nc.gpsimd.iota(iota_i[:], pattern=[[1, P]], base=0,
                           channel_multiplier=0)
            rowid_i = cst.tile([P, P], I32)
            nc.gpsimd.iota(rowid_i[:], pattern=[[0, P]], base=0,
                           channel_multiplier=1)
            ident = cst.tile([P, P], BF16)
            nc.vector.tensor_tensor(ident[:], iota_i[:], rowid_i[:],
                                    op=mybir.AluOpType.is_equal)

            # ---- inputs for the edge phase ----
            idx_s = cst.tile([P, NIDXCOLS], I16)
            nc.sync.dma_start(out=idx_s[:], in_=idx_d[:])

            # ---- phase W head: tanh projections (issued before phase A so
            #      only two scalar-queue ops precede A's PSUM copies; the
            #      DVE-heavy tail is issued after A and overlaps it) ----
            def ts_mul(out, in0, j):
                nc.vector.tensor_scalar_mul(out, in0, spar[:, j:j + 1])

            wkd = [cst.tile([P, 2 * TCE], BF16, name=f"wkd{k}", tag=f"wd{k}")
                   for k in range(K)]
            mskd_s = cst.tile([P, 2 * TCE], BF16)
            nc.sync.dma_start(out=mskd_s[:], in_=msk_d[:])
            with tc.tile_pool(name="wprep", bufs=1) as wpp:
                psa_s = wpp.tile([P, TCE], F32, tag="psa")
                nc.sync.dma_start(out=psa_s[:], in_=psa_d[:])
                psb_s = wpp.tile([P, TCE], F32, tag="psb")
                nc.sync.dma_start(out=psb_s[:], in_=psb_d[:])
                pa = wpp.tile([P, TCE], F32, tag="pa")
                pb = wpp.tile([P, TCE], F32, tag="pb")
                m1 = wpp.tile([P, TCE], F32, tag="m1")
                m2 = wpp.tile([P, TCE], F32, tag="m2")
                m3 = wpp.tile([P, TCE], F32, tag="m3")
                m4 = wpp.tile([P, TCE], F32, tag="m4")
                ts_mul(m1[:], psa_s[:], 0)
                ts_mul(m2[:], psb_s[:], 1)
                nc.vector.tensor_add(m1[:], m1[:], m2[:])
                ts_mul(m3[:], psa_s[:], 2)
                ts_mul(m4[:], psb_s[:], 3)
                nc.vector.tensor_add(m3[:], m3[:], m4[:])
                nc.scalar.activation(pa[:], m1[:],
                                     mybir.ActivationFunctionType.Tanh,
                                     bias=spar[:, 4:5])
                nc.scalar.activation(pb[:], m3[:],
                                     mybir.ActivationFunctionType.Tanh,
                                     bias=spar[:, 5:6])

                # ---- phase A: hp half-table build (bf16 table) ----
                BLKT = 8
                nblk = -(-NTH // BLKT)
                with (
                    tc.tile_pool(name="xblk", bufs=3) as xblkp,
                    tc.tile_pool(name="hps", bufs=4) as hpsp,
                    tc.tile_pool(name="psA", bufs=4, space="PSUM") as psA,
                ):
                    for b in range(nblk):
                        t0 = b * BLKT
                        tn = min(BLKT, NTH - t0)
                        xblk = xblkp.tile([IN_C, BLKT * P], BF16, tag="xblk")
                        nc.sync.dma_start(out=xblk[:, :tn * P],
                                          in_=xT[:, t0 * P:(t0 + tn) * P])
                        hps = hpsp.tile([P, BLKT * ROWF], BF16, tag="hps")
                        for i0 in range(0, tn, 2):
                            pn = min(2, tn - i0)
                            pst = psA.tile([P, 2 * ROWF], F32)
                            for i in range(i0, i0 + pn):
                                nc.tensor.matmul(
                                    pst[:,
                                        (i - i0) * ROWF:(i - i0 + 1) * ROWF],
                                    lhsT=xblk[:, i * P:(i + 1) * P],
                                    rhs=wTs[:], start=True, stop=True)
                            nc.scalar.activation(
                                hps[:, i0 * ROWF:(i0 + pn) * ROWF],
                                pst[:, :pn * ROWF],
                                mybir.ActivationFunctionType.Copy)
                        r0 = t0 * P
                        nc.sync.dma_start(
                            out=hp_d[r0:r0 + tn * P, 0:ROWF].rearrange(
                                "(g p) f -> p g f", p=P),
                            in_=hps[:, :tn * ROWF].rearrange(
                                "p (g f) -> p g f", f=ROWF))

                # ---- phase W tail: quadratic form + exp (overlaps A) ----
                qa = wpp.tile([P, TCE], F32, tag="psa")
                qb = wpp.tile([P, TCE], F32, tag="psb")
                nc.vector.tensor_tensor(qa[:], pa[:], pa[:],
                                        op=mybir.AluOpType.mult)
                nc.vector.tensor_tensor(qb[:], pb[:], pb[:],
                                        op=mybir.AluOpType.mult)
                for k in range(K):
                    u1 = wpp.tile([P, TCE], F32, tag="m1", name=f"u1_{k}")
                    u2 = wpp.tile([P, TCE], F32, tag="m2", name=f"u2_{k}")
                    u3 = wpp.tile([P, TCE], F32, tag="m3", name=f"u3_{k}")
                    u4 = wpp.tile([P, TCE], F32, tag="m4", name=f"u4_{k}")
                    ts_mul(u1[:], pa[:], 9 + k)
                    ts_mul(u2[:], pb[:], 12 + k)
                    ts_mul(u3[:], qa[:], 15 + k)
                    ts_mul(u4[:], qb[:], 18 + k)
                    nc.vector.tensor_add(u1[:], u1[:], u2[:])
                    nc.vector.tensor_add(u3[:], u3[:], u4[:])
                    nc.vector.tensor_add(u1[:], u1[:], u3[:])
                    # exp with pair-duplicated output, then pad-mask fold
                    nc.scalar.activation(
                        wkd[k][:].rearrange("p (c two) -> p c two", two=2),
                        u1[:].to_broadcast((P, TCE, 2)),
                        mybir.ActivationFunctionType.Exp,
                        bias=spar[:, 6 + k:7 + k])
                    nc.vector.tensor_tensor(
                        out=wkd[k][:], in0=wkd[k][:], in1=mskd_s[:],
                        op=mybir.AluOpType.mult)

            # ---- phase B: gather + fold + k-sum + per-tile add-tree ----
            sum_tiles = {}
            gat_tiles = {}
            LOOKAHEAD = 6

            def prefetch_gather(b):
                """Issue the gather only (gpsimd queue) -- deep prefetch
                without putting gather-dependent folds in the DVE queue."""
                if b in gat_tiles or b >= nb:
                    return
                c0 = b * G
                gn = min(G, TCE - c0)
                gat = gatp.tile([P, G * ROWP], BF16, tag="gat",
                                name=f"gat{b}")
                gv = gat[:].rearrange("p (j f) -> p j f", f=ROWP)
                nsplit = 4 if (b < 4 or b >= nb - 3) and gn % 4 == 0 \
                    else (2 if gn % 2 == 0 else 1)
                qn = gn // nsplit
                for qq in range(nsplit):
                    nc.gpsimd.dma_gather(
                        out_ap=gv[:, qq * qn:(qq + 1) * qn, :],
                        in_ap=hp_d[:, :],
                        idxs_ap=idx_s[:, b * SB + qq * qn * P // 16:
                                      b * SB + (qq + 1) * qn * P // 16],
                        num_idxs=qn * P, num_idxs_reg=qn * P,
                        elem_size=ROWP, single_packet=False,
                        queue_num=(2 * b + qq) % 4)
                gat_tiles[b] = gat

            def issue_batch(b):
                if b in sum_tiles:
                    return sum_tiles[b]
                prefetch_gather(b)
                prefetch_gather(b + LOOKAHEAD)
                c0 = b * G
                gn = min(G, TCE - c0)
                gat = gat_tiles.pop(b)
                gat3 = gat[:].rearrange("p (j f) -> p j f", f=ROWP)
                mks = []
                for k in range(K):
                    mk = msgp.tile([P, G * OUT], BF16, tag=f"mk{k}",
                                   name=f"mk{k}_{b}")
                    nc.vector.tensor_tensor(
                        out=mk[:].rearrange("p (j f2 two) -> p j f2 two",
                                            f2=F2, two=2)[:, :gn, :, :],
                        in0=gat3[:, :gn, k * OUT:(k + 1) * OUT].rearrange(
                            "p j (f2 two) -> p j f2 two", two=2),
                        in1=wkd[k][:, 2 * c0:2 * (c0 + gn)].rearrange(
                            "p (c two) -> p c two", two=2).unsqueeze(
                            2).broadcast_to((P, gn, F2, 2)),
                        op=mybir.AluOpType.mult)
                    mks.append(mk)
                # k-sum: sum3[p, c, o] = mk0 + mk1 + mk2
                sum3 = msgp.tile([P, G * OUT], BF16, tag="sum3",
                                 name=f"sum3_{b}")
                nc.vector.tensor_add(sum3[:, :gn * OUT], mks[0][:, :gn * OUT],
                                     mks[1][:, :gn * OUT])
                nc.vector.tensor_add(sum3[:, :gn * OUT], sum3[:, :gn * OUT],
                                     mks[2][:, :gn * OUT])
                sum_tiles[b] = sum3
                return sum3

            def col_view(buf, a, n):
                return buf[:, a * OUT:(a + n) * OUT]

            def seg_tree(t, s0, s1, out_f32):
                """Tree-sum sum3 columns [s0,s1) of one batch.
                If out_f32 is not None, the final add writes it (f32) and
                returns None; otherwise returns a (P, OUT) bf16 part."""
                b = s0 // G
                sum3 = issue_batch(b)
                src, a, n = sum3, s0 - b * G, s1 - s0
                while n > 2:
                    h = n // 2
                    scr = redp.tile([P, (G // 2) * OUT], BF16, tag="scr",
                                    name=f"scr{t}_{s0}_{n}")
                    nc.vector.tensor_add(col_view(scr, 0, h),
                                         col_view(src, a, h),
                                         col_view(src, a + h, h))
                    if n % 2:
                        nc.vector.tensor_add(col_view(scr, 0, 1),
                                             col_view(scr, 0, 1),
                                             col_view(src, a + 2 * h, 1))
                    src, a, n = scr, 0, h
                if n == 2:
                    if out_f32 is not None:
                        nc.vector.tensor_add(out_f32[:],
                                             col_view(src, a, 1),
                                             col_view(src, a + 1, 1))
                        return None
                    part = redp.tile([P, OUT], BF16, tag="part",
                                     name=f"part{t}_{s0}")
                    nc.vector.tensor_add(part[:], col_view(src, a, 1),
                                         col_view(src, a + 1, 1))
                    return (part, 0)
                # n == 1
                if out_f32 is not None:
                    nc.vector.tensor_scalar(
                        out=out_f32[:], in0=col_view(src, a, 1),
                        scalar1=0.0, scalar2=None, op0=mybir.AluOpType.add)
                    return None
                return (src, a)

            for b in range(LOOKAHEAD):
                prefetch_gather(b)
            for t in range(T4):
                cb0, cb1 = colbase[t], colbase[t + 1]
                if t in pe_slot:
                    # PE path: PSUM[o, p] += sum3_col[p, o] via identity rhs
                    ps = psB.tile([OUT, P], F32, tag="peacc",
                                  name=f"pe{t}")
                    ncols = cb1 - cb0
                    for ci in range(ncols):
                        col = cb0 + ci
                        b = col // G
                        sum3 = issue_batch(b)
                        nc.tensor.matmul(
                            ps[:], lhsT=col_view(sum3, col - b * G, 1),
                            rhs=ident[:], start=(ci == 0),
                            stop=(ci == ncols - 1))
                    pe_sb = hrowp.tile([OUT, P], F32, tag="pesb",
                                       name=f"pesb{t}")
                    nc.scalar.activation(pe_sb[:], ps[:],
                                         mybir.ActivationFunctionType.Copy)
                    j = pe_slot[t]
                    nc.sync.dma_start(out=houtpe_d[:, j * P:(j + 1) * P],
                                      in_=pe_sb[:])
                    continue
                hrow = hrowp.tile([P, OUT], F32, tag="hrow",
                                  name=f"hrow{t}")
                segs = []
                s = cb0
                while s < cb1:
                    e = min(cb1, (s // G + 1) * G)
                    segs.append((s, e))
                    s = e
                if not segs:
                    nc.vector.memset(hrow[:], 0.0)
                elif len(segs) == 1:
                    seg_tree(t, segs[0][0], segs[0][1], hrow)
                else:
                    parts = [seg_tree(t, s0, s1, None) for s0, s1 in segs]
                    buf0, a0 = parts[0]
                    buf1, a1 = parts[1]
                    nc.vector.tensor_add(hrow[:], col_view(buf0, a0, 1),
                                         col_view(buf1, a1, 1))
                    for bufx, ax in parts[2:]:
                        nc.vector.tensor_add(hrow[:], hrow[:],
                                             col_view(bufx, ax, 1))
                nc.sync.dma_start(out=hout_d[t * P:(t + 1) * P, :],
                                  in_=hrow[:])

    nc.compile()
    return nc


# ----------------------------------------------------------------------------
# Full model runner
# ----------------------------------------------------------------------------

_KERNEL_CACHE = {}


def _get_kernel(key, builder):
    if key not in _KERNEL_CACHE:
        _KERNEL_CACHE[key] = builder()
    return _KERNEL_CACHE[key]


def _run_layer(layer_tag, md, xfull_T, fcT, par, bias, IN_C, OUT,
               n_cores, trace):
    """xfull_T: (IN_C, NPAD) bf16 full-node feature table (feature-major).
    Returns (NPAD, OUT) f32 node outputs (bias added)."""
    T4, HALF, QN = md["T4"], md["HALF"], md["QN"]
    C_uni, G = md["C_uni"], md["G"]
    nc = _get_kernel((layer_tag, T4, HALF, C_uni, IN_C, OUT, G, n_cores),
                     lambda: build_layer_kernel(T4, HALF, C_uni, IN_C, OUT,
                                                3, G, n_cores))
    in_maps = []
    for c in range(n_cores):
        h = c // 4
        in_maps.append(dict(
            xT=np.ascontiguousarray(xfull_T[:, h * HALF:(h + 1) * HALF]),
            wT=fcT,
            idx_w=md["idx_w"][c], maskd_t=md["maskd_t"][c],
            ps_a=md["ps_a"][c], ps_b=md["ps_b"][c],
            par=par))
    res = bass_utils.run_bass_kernel_spmd(
        nc, in_maps, core_ids=list(range(n_cores)), trace=trace)

    NPAD = 2 * HALF
    pe_tiles = [t for t in range(T4) if C_uni[t] > 0 and t % 3 < 2]
    out = np.zeros((NPAD, OUT), np.float32)
    order = md["order"]
    for c in range(8):
        harr = np.array(res.results[c]["h_out"])
        hpe = res.results[c]["h_out_pe"]
        for j, t in enumerate(pe_tiles):
            harr[t * P:(t + 1) * P] = hpe[:, j * P:(j + 1) * P].T
        base = (c % 4) * QN
        out[base + order[c]] += harr
    out += bias.reshape(1, OUT)
    return out, res.exec_time_ns


def run_monet(inputs, n_cores=8, G=16, trace=False):
    feat = np.asarray(inputs["feat"], np.float32)
    pseudo = np.asarray(inputs["pseudo"], np.float32)
    src = np.asarray(inputs["src"], np.int32)
    dst = np.asarray(inputs["dst"], np.int32)
    N, IN_F = feat.shape
    HID = np.asarray(inputs["fc0"]).shape[0] // 3
    OUTF = np.asarray(inputs["fc1"]).shape[0] // 3

    md = build_ell_metadata(src, dst, pseudo, N, n_cores, G)
    NPAD = md["NT"] * P

    featT = np.zeros((IN_F, NPAD), NPBF)
    featT[:, :N] = feat.T.astype(NPBF)
    fc0T = np.ascontiguousarray(
        np.asarray(inputs["fc0"], np.float32).T.astype(NPBF))
    fc1T = np.ascontiguousarray(
        np.asarray(inputs["fc1"], np.float32).T.astype(NPBF))
    par0 = pack_params(np.asarray(inputs["pp0_w"], np.float32),
                       np.asarray(inputs["pp0_b"], np.float32),
                       np.asarray(inputs["mu0"], np.float32),
                       np.asarray(inputs["inv_sigma0"], np.float32))
    par1 = pack_params(np.asarray(inputs["pp1_w"], np.float32),
                       np.asarray(inputs["pp1_b"], np.float32),
                       np.asarray(inputs["mu1"], np.float32),
                       np.asarray(inputs["inv_sigma1"], np.float32))
    b0 = np.asarray(inputs["b0"], np.float32)
    b1 = np.asarray(inputs["b1"], np.float32)

    h0, t0 = _run_layer("l0v4", md, featT, fc0T, par0, b0, IN_F, HID,
                        n_cores, trace)

    hT = np.ascontiguousarray(h0.T.astype(NPBF))
    h1, t1 = _run_layer("l1v4", md, hT, fc1T, par1, b1, HID, OUTF,
                        n_cores, trace)

    out = np.ascontiguousarray(h1[:N])
    perf = dict(l0_ns=t0, l1_ns=t1)
    return out, perf


# ----------------------------------------------------------------------------
# Harness entry: full inputs in, full output out
# ----------------------------------------------------------------------------

def kernel(**inputs):
    out, _ = run_monet(inputs)
    return out.astype(np.float32)
